# revision 1
# baseline (speedup 1.0000x reference)
"""BinaryOneToManyMatcher (nms_detection) Trainium2 Bass kernel.

Computes, for B=128 images with Q=1000 predicted boxes and G=300 GT boxes:
  score = sigmoid(pred_logits)            [B,Q]
  iou   = pairwise IoU(pred, tgt)         [B,Q,G]
  gt    = score * iou * (iou > 0.4)       [B,Q,G]
  vals, idxs = top_k(gt over Q, k=4); mask = vals > 0

Sharding: pure data parallel, 16 images per NeuronCore across 8 cores.

Per-core layout: for each image, G on partitions (chunks of <=128) and Q on
the free dim (1000 wide).  Per-query rows (x1,y1,x2,y2,area,score) are
broadcast across partitions via PE ones-matmul (bit-exact); per-target
values are [P,1]
per-partition scalars, so the whole IoU chain runs as fused
tensor_scalar / scalar_tensor_tensor ops on the Vector engine.

Top-4 uses the DVE Max8 instruction (top-8 per partition, descending) +
MaxIndex.  A strictly-decreasing per-q bias of scale 2^-40 is added to the
masked scores so every value in a row is distinct; this makes tie handling
exact: zero entries (invalid pairs) sort by ascending q, matching
jax.lax.top_k's lowest-index-first tie rule, and the bias is far below the
minimum positive score gap so positive ordering is unchanged.
"""

from contextlib import ExitStack

import numpy as np

import concourse.bass as bass
import concourse.tile as tile
from concourse import bacc, mybir
from concourse.bass_utils import run_bass_kernel_spmd

B, Q, G, K = 128, 1000, 300, 4
NCORES = 8
BPC = B // NCORES  # images per core

F32 = mybir.dt.float32
I32 = mybir.dt.int32
U32 = mybir.dt.uint32
U8 = mybir.dt.uint8
Op = mybir.AluOpType

BIAS_SCALE = float(2.0**-40)  # per-q tie-break bias scale
POS_THRESH = 1e-6  # separates real positives (>=3e-3) from bias values (<1e-9)


def _register_wsub():
    """Custom DVE op: out = min(in0, s0) - max(in1, s1) in one pass.

    Computes the overlap width rb-lt of the IoU kernel (normally a
    tensor_scalar max + a fused min/subtract = 2 DVE passes) in a single
    full-rate instruction.  Rounding matches the reference exactly: min/max
    are exact, one rounded subtract.
    """
    from concourse import dve_ops
    from concourse.dve_spec import Spec, Src0, Src1, C0, C1, minn, maxx, lower
    from concourse.dve_uop import DveOpSpec

    for op in dve_ops.OPS:
        if op.name == "WSUB_ANT":
            return op

    spec = Spec(
        body=minn(Src0, C0) - maxx(Src1, C1),
        reference=lambda in0, in1, s0, s1, imm2: (
            np.minimum(in0.astype(np.float32), s0) - np.maximum(in1, s1)
        ).astype(np.float32),
    )
    shas = {}
    for ver in ("v3", "v4"):
        try:
            uops = lower(spec, ver=ver)
            shas[ver] = DveOpSpec(
                name="WSUB_ANT", opcode=0, uops=uops, rd1_en=True
            ).sha(ver)
        except Exception:
            pass
    op = dve_ops.DveOp("WSUB_ANT", spec, subdim=False, uops_sha=shas)
    dve_ops.OPS.append(op)
    dve_ops.CUSTOM_DVE_SPECS[op.name] = spec
    dve_ops._SUB_OPCODE_FOR_NAME[op.name] = (
        max(dve_ops._SUB_OPCODE_FOR_NAME.values()) + 1
    )
    assert dve_ops._SUB_OPCODE_FOR_NAME[op.name] < 0x20
    return op


def _build_kernel(reps=1):
    try:
        wsub = _register_wsub()
    except Exception:
        wsub = None  # fall back to the unfused 2-op form
    nc = bacc.Bacc("TRN2", target_bir_lowering=False, debug=False,
                   num_devices=NCORES)

    pl = nc.dram_tensor("pred_logits", [BPC, Q, 1], F32, kind="ExternalInput").ap()
    pb = nc.dram_tensor("pred_boxes", [BPC, Q, 4], F32, kind="ExternalInput").ap()
    tb = nc.dram_tensor("tgt_boxes", [BPC, G, 4], F32, kind="ExternalInput").ap()

    vals_o = nc.dram_tensor("vals", [BPC, G, K], F32, kind="ExternalOutput").ap()
    idxs_o = nc.dram_tensor("idxs", [BPC, G, K], I32, kind="ExternalOutput").ap()
    mask_o = nc.dram_tensor("mask", [BPC, G, K], U8, kind="ExternalOutput").ap()

    NQ = BPC * Q          # 16000 query slots across the core's images
    QP = NQ // 128        # 125 queries per partition in phase-0 layout

    with tile.TileContext(nc) as tc, ExitStack() as ctx:
        dram = ctx.enter_context(tc.tile_pool(name="dram", bufs=1, space="DRAM"))
        const = ctx.enter_context(tc.tile_pool(name="const", bufs=1))
        prep = ctx.enter_context(tc.tile_pool(name="prep", bufs=1))
        rows = ctx.enter_context(tc.tile_pool(name="rows", bufs=2))
        lines = ctx.enter_context(tc.tile_pool(name="lines", bufs=4))
        work = ctx.enter_context(tc.tile_pool(name="work", bufs=2))
        tiny = ctx.enter_context(tc.tile_pool(name="tiny", bufs=2))
        outp = ctx.enter_context(tc.tile_pool(name="outp", bufs=2))
        psum = ctx.enter_context(tc.tile_pool(name="psum", bufs=4, space="PSUM"))

        # ---- phase 0: build rowpack = [px1,py1,px2,py2,area,score] per image
        # (contiguous per-image 6*Q lines; broadcast later via PE ones-matmul,
        # NOT stride-0 DMA — partition-broadcast DMA is descriptor-bound at
        # ~96us per [128,1000] broadcast on HW)
        rowpack_d = dram.tile([BPC * 6 * Q], F32, tag="rowpack")
        rowview = rowpack_d[:].rearrange("(b c q) -> b c q", c=6, q=Q)
        PH = Q // QP  # partitions per image in phase-0 layout (8)

        def pack_row(j, tile_view):
            # tile_view: [128, QP] SBUF, partition 8b+ph = queries of image b
            for bb in range(BPC):
                nc.sync.dma_start(
                    rowview[bb, j, :].rearrange("(ph r) -> ph r", ph=PH),
                    tile_view[bb * PH:(bb + 1) * PH, :],
                )

        # packed pred boxes: partition p holds queries [QP*p, QP*p+QP), 4 coords
        pbt = prep.tile([128, QP * 4], F32, tag="pbt")
        src = pb.rearrange("b q c -> (b q c)").rearrange("(p x) -> p x", p=128)
        nc.sync.dma_start(pbt[:], src)
        pv = pbt[:].rearrange("p (r c) -> p r c", c=4)
        for c in range(4):
            pack_row(c, pv[:, :, c])
        dx = prep.tile([128, QP], F32, tag="dx")
        dy = prep.tile([128, QP], F32, tag="dy")
        pa = prep.tile([128, QP], F32, tag="pa")
        nc.vector.tensor_tensor(dx[:], pv[:, :, 2], pv[:, :, 0], Op.subtract)
        nc.vector.tensor_tensor(dy[:], pv[:, :, 3], pv[:, :, 1], Op.subtract)
        nc.vector.tensor_tensor(pa[:], dx[:], dy[:], Op.mult)
        pack_row(4, pa[:])

        # target areas for all images: tgt boxes flat 16*300*4 = 19200 = 96*200
        ta_d = dram.tile([BPC * G], F32, tag="ta_d")
        tbt = prep.tile([96, 200], F32, tag="tbt")
        nc.sync.dma_start(
            tbt[:], tb.rearrange("b g c -> (b g c)").rearrange("(p x) -> p x", p=96)
        )
        tv = tbt[:].rearrange("p (r c) -> p r c", c=4)
        tdx = prep.tile([96, 50], F32, tag="tdx")
        tdy = prep.tile([96, 50], F32, tag="tdy")
        tar = prep.tile([96, 50], F32, tag="tar")
        nc.vector.tensor_tensor(tdx[:], tv[:, :, 2], tv[:, :, 0], Op.subtract)
        nc.vector.tensor_tensor(tdy[:], tv[:, :, 3], tv[:, :, 1], Op.subtract)
        nc.vector.tensor_tensor(tar[:], tdx[:], tdy[:], Op.mult)
        nc.sync.dma_start(ta_d[:].rearrange("(p r) -> p r", p=96), tar[:])

        # sigmoid(x) = 1 / (1 + exp(-x)); exp on ScalarE, exact-ish recip on DVE
        lg = prep.tile([128, QP], F32, tag="lg")
        nc.sync.dma_start(
            lg[:], pl.rearrange("b q c -> (b q c)").rearrange("(p x) -> p x", p=128)
        )
        ex = prep.tile([128, QP], F32, tag="ex")
        nc.scalar.activation(ex[:], lg[:], mybir.ActivationFunctionType.Exp,
                             scale=-1.0)
        w1 = prep.tile([128, QP], F32, tag="w1")
        nc.vector.tensor_scalar(w1[:], ex[:], 1.0, None, Op.add)
        sc = prep.tile([128, QP], F32, tag="sc")
        scr = prep.tile([128, QP], F32, tag="scr")
        nc.vector.reciprocal_approx_accurate(sc[:], w1[:], scr[:])
        pack_row(5, sc[:])

        # ones row for PE-based partition broadcast
        ones = const.tile([1, 128], F32, tag="ones")
        nc.vector.memset(ones[:], 1.0)
        # epsilon row for the union + 1e-7 add on gpsimd
        epsr = const.tile([128, Q], F32, tag="epsr")
        nc.vector.memset(epsr[:], 1e-7)
        c04r = const.tile([128, Q], F32, tag="c04r")
        nc.vector.memset(c04r[:], 0.4)

        # ---- tie-break bias row: (Q - q) * 2^-40, identical on all partitions
        bias_i = const.tile([128, Q], I32, tag="bias_i")
        nc.gpsimd.iota(bias_i[:], pattern=[[-1, Q]], base=Q, channel_multiplier=0)
        bias_f = const.tile([128, Q], F32, tag="bias_f")
        nc.vector.tensor_scalar(bias_f[:], bias_i[:], BIAS_SCALE, None, Op.mult)

        # ---- main loop: per image, per g-chunk ----
        for b in [bb for _ in range(reps) for bb in range(BPC)]:
            r_px1 = rows.tile([128, Q], F32, tag="px1")
            r_py1 = rows.tile([128, Q], F32, tag="py1")
            r_px2 = rows.tile([128, Q], F32, tag="px2")
            r_py2 = rows.tile([128, Q], F32, tag="py2")
            r_pa = rows.tile([128, Q], F32, tag="pa")
            r_sc = rows.tile([128, Q], F32, tag="sc")
            # per-row [1,Q] line DMAs (partition 0, contiguous), then PE
            # ones-matmul broadcast (bit-exact: 1.0*x) + ScalarE copies
            HB = 500  # psum bank-sized matmul piece (N<=512)
            for j, rt in enumerate((r_px1, r_py1, r_px2, r_py2, r_pa, r_sc)):
                line = lines.tile([1, Q], F32, tag="line")
                nc.sync.dma_start(
                    line[:],
                    rowpack_d[(b * 6 + j) * Q:(b * 6 + j + 1) * Q]
                    .rearrange("(a x) -> a x", a=1),
                )
                pt = psum.tile([128, 1024], F32, tag="pt")
                for h in range(Q // HB):
                    nc.tensor.matmul(pt[:, h * 512:h * 512 + HB], ones[:],
                                     line[0:1, h * HB:(h + 1) * HB],
                                     start=True, stop=True)
                # r_sc is stored as score/2 (exact) for the sign-trick mask
                sc_half = 0.5 if rt is r_sc else 1.0
                nc.scalar.activation(
                    rt[:].rearrange("p (h x) -> p h x", h=2),
                    pt[:].rearrange("p (h x) -> p h x", h=2)[:, :, 0:HB],
                    mybir.ActivationFunctionType.Copy, scale=sc_half)

            # per-image collectors for the top-8 results of the 3 g-chunks,
            # so the tiny threshold/mask epilogue runs once per image
            v8s = outp.tile([128, 24], F32, tag="v8s")
            i8s = outp.tile([128, 24], U32, tag="i8s")
            nc.gpsimd.memset(v8s[:], 0.0)
            for ci, g0 in enumerate(range(0, G, 128)):
                P = min(128, G - g0)

                tsc = tiny.tile([P, 4], F32, tag="tsc")
                nc.sync.dma_start(tsc[:], tb[b, g0:g0 + P, :])
                tx1, ty1 = tsc[:, 0:1], tsc[:, 1:2]
                tx2, ty2 = tsc[:, 2:3], tsc[:, 3:4]
                ta = tiny.tile([P, 1], F32, tag="ta")
                nc.sync.dma_start(
                    ta[:],
                    ta_d[b * G + g0:b * G + g0 + P].rearrange("(p x) -> p x", x=1),
                )

                # w/h pre-relu overlap widths, one fused custom op each:
                # wxr = min(px2, tx2) - max(px1, tx1)
                def wsub_op(out_t, hi_row, lo_row, hi_s, lo_s, ltag):
                    if wsub is not None:
                        nc.vector._custom_dve(wsub, out=out_t[:], in0=hi_row,
                                              in1=lo_row, s0=hi_s, s1=lo_s)
                    else:
                        lt = work.tile([P, Q], F32, tag=ltag)
                        nc.vector.tensor_scalar(lt[:], lo_row, lo_s, None, Op.max)
                        nc.vector.scalar_tensor_tensor(out_t[:], hi_row, hi_s,
                                                       lt[:], Op.min, Op.subtract)

                wxr = work.tile([P, Q], F32, tag="A")
                wsub_op(wxr, r_px2[:P], r_px1[:P], tx2, tx1, "F")
                wyr = work.tile([P, Q], F32, tag="B")
                wsub_op(wyr, r_py2[:P], r_py1[:P], ty2, ty1, "F")
                # inter = relu(wxr) * wyr  (sign-exact; == ref where it matters)
                inter = work.tile([P, Q], F32, tag="G")
                nc.vector.scalar_tensor_tensor(inter[:], wxr[:], 0.0, wyr[:],
                                               Op.max, Op.mult)
                # U = (pa + ta) - inter ; Up = U + 1e-7
                U = work.tile([P, Q], F32, tag="C")
                nc.vector.scalar_tensor_tensor(U[:], r_pa[:P], ta[:, 0:1], inter[:],
                                               Op.add, Op.subtract)
                Up = work.tile([P, Q], F32, tag="D")
                nc.gpsimd.tensor_tensor(Up[:], U[:], epsr[:P], Op.add)
                import os as _os2
                _fma_negd = _os2.environ.get("KB_FMA_NEGD", "0") == "1"
                # negd = 0.4*Up - inter  (valid <=> negd < 0); gpsimd arith
                # (rounding identical to the fused DVE form), DVE fallback
                # selectable for A/B because HW gpsimd throughput is uncertain
                if _fma_negd:
                    # one ScalarE fma replaces Up*0.4 (1 ULP vs two-rounding
                    # form; validated exactly against the dataset in sim)
                    n1 = work.tile([P, Q], F32, tag="J")
                    nc.scalar.activation(n1[:], U[:],
                                         mybir.ActivationFunctionType.Copy,
                                         scale=0.4, bias=float(0.4 * 1e-7))
                else:
                    n1 = work.tile([P, Q], F32, tag="J")
                    nc.gpsimd.tensor_tensor(n1[:], Up[:], c04r[:P], Op.mult)
                negd = work.tile([P, Q], F32, tag="E")
                nc.gpsimd.tensor_tensor(negd[:], n1[:], inter[:], Op.subtract)
                # R ~= 1/Up to ~2 ULP
                R = work.tile([P, Q], F32, tag="C")
                rs = work.tile([P, Q], F32, tag="F")
                nc.vector.reciprocal_approx_accurate(R[:], Up[:], rs[:])
                # m3 = (negd < 0) * ((inter * R) * score) + bias
                # plain tensor_tensor muls/adds run on gpsimd: DVE 1x ops
                # never contend with the shared port, so this is free overlap
                m1 = work.tile([P, Q], F32, tag="A")
                nc.gpsimd.tensor_tensor(m1[:], inter[:], R[:], Op.mult)
                # t1 = m1 * (score/2); valid mask via sign(negd) on ScalarE:
                # m2 = t1 - t1*sgn = 2*t1*valid = valid*m1*score, all exact
                # (x2 / /2 are exact; sgn in {-1,+1}, negd==0 cannot occur)
                t1 = work.tile([P, Q], F32, tag="B")
                nc.gpsimd.tensor_tensor(t1[:], m1[:], r_sc[:P], Op.mult)
                sgn = work.tile([P, Q], F32, tag="H")
                nc.scalar.activation(sgn[:], negd[:],
                                     mybir.ActivationFunctionType.Sign)
                u1 = work.tile([P, Q], F32, tag="I")
                nc.gpsimd.tensor_tensor(u1[:], t1[:], sgn[:], Op.mult)
                m2 = work.tile([P, Q], F32, tag="H")
                nc.gpsimd.tensor_tensor(m2[:], t1[:], u1[:], Op.subtract)
                m3 = work.tile([P, Q], F32, tag="E")
                nc.gpsimd.tensor_tensor(m3[:], m2[:], bias_f[:P], Op.add)

                v8 = v8s[0:P, 8 * ci:8 * ci + 8]
                nc.vector.max(v8, m3[:])
                nc.vector.max_index(i8s[0:P, 8 * ci:8 * ci + 8], v8, m3[:])

            # batched epilogue: exact zeros for padding slots + bool mask
            v8v = v8s[:].rearrange("p (c e) -> p c e", e=8)[:, :, 0:K]
            v4b = outp.tile([128, 3 * K], F32, tag="v4b")
            nc.vector.scalar_tensor_tensor(
                v4b[:].rearrange("p (c e) -> p c e", e=K), v8v, POS_THRESH,
                v8v, Op.is_gt, Op.mult)
            mkb = outp.tile([128, 3 * K], U8, tag="mkb")
            nc.vector.tensor_scalar(
                mkb[:].rearrange("p (c e) -> p c e", e=K), v8v, POS_THRESH,
                None, Op.is_gt)
            for ci, g0 in enumerate(range(0, G, 128)):
                P = min(128, G - g0)
                nc.sync.dma_start(vals_o[b, g0:g0 + P, :],
                                  v4b[0:P, K * ci:K * ci + K])
                nc.sync.dma_start(idxs_o[b, g0:g0 + P, :],
                                  i8s[0:P, 8 * ci:8 * ci + K].bitcast(I32))
                nc.sync.dma_start(mask_o[b, g0:g0 + P, :],
                                  mkb[0:P, K * ci:K * ci + K])

    nc.compile()
    return nc


_NC = None


def _get_nc():
    global _NC
    if _NC is None:
        _NC = _build_kernel()
    return _NC


def run(pred_logits, pred_boxes_xyxy, tgt_boxes_xyxy, **spmd_kwargs):
    nc = _get_nc()
    pred_logits = np.ascontiguousarray(np.asarray(pred_logits, dtype=np.float32))
    pred_boxes = np.ascontiguousarray(np.asarray(pred_boxes_xyxy, dtype=np.float32))
    tgt_boxes = np.ascontiguousarray(np.asarray(tgt_boxes_xyxy, dtype=np.float32))
    in_maps = [
        {
            "pred_logits": pred_logits[c * BPC:(c + 1) * BPC],
            "pred_boxes": pred_boxes[c * BPC:(c + 1) * BPC],
            "tgt_boxes": tgt_boxes[c * BPC:(c + 1) * BPC],
        }
        for c in range(NCORES)
    ]
    res = run_bass_kernel_spmd(nc, in_maps, list(range(NCORES)), **spmd_kwargs)
    vals = np.concatenate([res.results[c]["vals"] for c in range(NCORES)], axis=0)
    idxs = np.concatenate([res.results[c]["idxs"] for c in range(NCORES)], axis=0)
    mask = np.concatenate([res.results[c]["mask"] for c in range(NCORES)], axis=0)
    return (vals, idxs.astype(np.int32), mask.astype(bool)), res


def kernel(pred_logits, pred_boxes_xyxy, tgt_boxes_xyxy):
    (vals, idxs, mask), _ = run(pred_logits, pred_boxes_xyxy, tgt_boxes_xyxy)
    return vals, idxs, mask



# revision 7
# speedup vs baseline: 3.8687x; 3.8687x over previous
"""BinaryOneToManyMatcher (nms_detection) Trainium2 Bass kernel.

Computes, for B=128 images with Q=1000 predicted boxes and G=300 GT boxes:
  score = sigmoid(pred_logits)            [B,Q]
  iou   = pairwise IoU(pred, tgt)         [B,Q,G]
  gt    = score * iou * (iou > 0.4)       [B,Q,G]
  vals, idxs = top_k(gt over Q, k=4); mask = vals > 0

Sharding: pure data parallel, 16 images per NeuronCore across 8 cores.

Per-core layout: per image, G on partitions in 3 chunks of 100 and Q on the
free dim (1000 wide).  Per-query rows (x1,y1,x2,y2,area+eps,score) are
broadcast across partitions via PE ones-matmul (bit-exact); per-target
values are [P,1] per-partition scalars.

v2 changes vs the original baseline (1.04ms HW):
 - DMA count per core cut from 436 to ~25.  Each DMA serializes ~0.6us on
   the shared HWDGE unit plus ~1us of descriptor/semaphore latency, so the
   baseline spent ~450us on DMA.  Now: queries are packed once into a
   per-image "line" layout [16,6000] via one SBUF->SBUF reorg DMA, each
   image stages its 6 rows with ONE single-descriptor DMA, target boxes
   arrive in one batched DMA, target areas are computed on-chip, and the
   three outputs are written with one batched DMA each at the end.
 - Mask chain restructured: valid = inter > 0.4*Up computed as a single
   fused custom-DVE select (MASKNUM) instead of 4 gpsimd + 1 act ops.
 - Epilogue (zeroing + bool mask) runs once per core instead of per image.

Top-4 uses the DVE Max8 instruction (top-8 per partition, descending) +
MaxIndex.  A strictly-decreasing per-q bias of scale 2^-40 is added to the
masked scores so zero entries (invalid pairs) sort by ascending q, matching
jax.lax.top_k's lowest-index-first tie rule; the bias is far below the
minimum positive score gap so positive ordering is unchanged.
"""

import os
from contextlib import ExitStack

import numpy as np

import concourse.bass as bass
import concourse.tile as tile
from concourse import bacc, mybir
from concourse.bass_utils import run_bass_kernel_spmd

B, Q, G, K = 128, 1000, 300, 4
NCORES = 8
BPC = B // NCORES  # images per core
PCH = 100          # partitions per g-chunk (3 chunks of 100 = G)
NCH = G // PCH

F32 = mybir.dt.float32
I32 = mybir.dt.int32
U32 = mybir.dt.uint32
U8 = mybir.dt.uint8
Op = mybir.AluOpType

BIAS_SCALE = float(2.0**-40)  # per-q tie-break bias scale
POS_THRESH = 1e-6  # separates real positives (>=3e-3) from bias values (<1e-9)


def _register_dve_ops():
    """Custom DVE ops, each one full-rate pass:

    WSUB_ANT:   out = min(in0, s0) - max(in1, s1)       (overlap width)
    MASKNUM_ANT: out = in0 if in0 > in1*s0 else 0       (masked numerator)
    """
    from concourse import dve_ops
    from concourse.dve_spec import (
        Spec, Src0, Src1, C0, C1, Zero, minn, maxx, select, relu, lower,
    )
    from concourse.dve_uop import DveOpSpec

    def reg(name, spec):
        for op in dve_ops.OPS:
            if op.name == name:
                return op
        shas = {}
        for ver in ("v3", "v4"):
            try:
                uops = lower(spec, ver=ver)
                shas[ver] = DveOpSpec(
                    name=name, opcode=0, uops=uops, rd1_en=True
                ).sha(ver)
            except Exception:
                pass
        op = dve_ops.DveOp(name, spec, subdim=False, uops_sha=shas)
        dve_ops.OPS.append(op)
        dve_ops.CUSTOM_DVE_SPECS[op.name] = spec
        dve_ops._SUB_OPCODE_FOR_NAME[op.name] = (
            max(dve_ops._SUB_OPCODE_FOR_NAME.values()) + 1
        )
        assert dve_ops._SUB_OPCODE_FOR_NAME[op.name] < 0x20
        return op

    wsub = reg("WSUB_ANT", Spec(
        body=minn(Src0, C0) - maxx(Src1, C1),
        reference=lambda in0, in1, s0, s1, imm2: (
            np.minimum(in0.astype(np.float32), s0) - np.maximum(in1, s1)
        ).astype(np.float32),
    ))
    wsubrelu = reg("WSUBRELU_ANT", Spec(
        body=relu(minn(Src0, C0) - maxx(Src1, C1)),
        reference=lambda in0, in1, s0, s1, imm2: np.maximum(
            np.minimum(in0.astype(np.float32), s0) - np.maximum(in1, s1), 0.0
        ).astype(np.float32),
    ))
    masknum = reg("MASKNUM_ANT", Spec(
        body=select(Src0 > Src1 * C0, Src0, Zero),
        reference=lambda in0, in1, s0, s1, imm2: np.where(
            in0 > (in1 * s0).astype(np.float32), in0, np.float32(0.0)
        ).astype(np.float32),
    ))
    return wsub, wsubrelu, masknum


def _build_kernel(reps=1):
    wsub, wsubrelu, masknum = _register_dve_ops()
    from concourse.dve_ops import RECIPROCAL_APPROX_NR

    kb_nm = os.environ.get("KB_NM", "dve")
    kb_nr = os.environ.get("KB_NR", "dve")
    kb_up = os.environ.get("KB_UP", "pool")
    kb_m1 = os.environ.get("KB_M1", "pool")
    kb_t1 = os.environ.get("KB_T1", "pool")
    kb_m3 = os.environ.get("KB_M3", "pool")
    kb_out = os.environ.get("KB_OUT", "batch")

    nc = bacc.Bacc("TRN2", target_bir_lowering=False, debug=False,
                   num_devices=NCORES)

    pl = nc.dram_tensor("pred_logits", [BPC, Q, 1], F32, kind="ExternalInput").ap()
    pb = nc.dram_tensor("pred_boxes", [BPC, Q, 4], F32, kind="ExternalInput").ap()
    tb = nc.dram_tensor("tgt_boxes", [BPC, G, 4], F32, kind="ExternalInput").ap()

    vals_o = nc.dram_tensor("vals", [BPC, G, K], F32, kind="ExternalOutput").ap()
    idxs_o = nc.dram_tensor("idxs", [BPC, G, K], I32, kind="ExternalOutput").ap()
    mask_o = nc.dram_tensor("mask", [BPC, G, K], U8, kind="ExternalOutput").ap()

    PH = 8          # partitions per image in the packed query layout
    QP = Q // PH    # 125 queries per partition

    with tile.TileContext(nc) as tc, ExitStack() as ctx:
        const = ctx.enter_context(tc.tile_pool(name="const", bufs=1))
        prep = ctx.enter_context(tc.tile_pool(name="prep", bufs=1))
        persist = ctx.enter_context(tc.tile_pool(name="persist", bufs=1))
        stagep = ctx.enter_context(tc.tile_pool(name="stage", bufs=1))
        rows = ctx.enter_context(tc.tile_pool(name="rows", bufs=2))
        work = ctx.enter_context(tc.tile_pool(name="work", bufs=2))
        psum = ctx.enter_context(tc.tile_pool(name="psum", bufs=4, space="PSUM"))

        # ---- constants
        ones = const.tile([1, 128], F32, tag="ones")
        nc.vector.memset(ones[:], 1.0)
        # tie-break bias row: (Q - q) * 2^-40, identical on all partitions
        bias_i = const.tile([128, Q], I32, tag="bias_i")
        nc.gpsimd.iota(bias_i[:], pattern=[[-1, Q]], base=Q, channel_multiplier=0)
        bias_f = const.tile([128, Q], F32, tag="bias_f")
        nc.vector.tensor_scalar(bias_f[:], bias_i[:], BIAS_SCALE, None, Op.mult)
        if kb_nm == "pool":
            c04r = const.tile([128, Q], F32, tag="c04r")
            nc.vector.memset(c04r[:], 0.4)
        if kb_nr == "pool":
            c2r = const.tile([128, Q], F32, tag="c2r")
            nc.vector.memset(c2r[:], 2.0)

        # ---- prep: pack per-query rows into per-image lines [16, 6000]
        # lines_all[b, :] = [px1|py1|px2|py2 (ph,c,r packed), pa+eps, score]
        lines_all = persist.tile([BPC, 6 * Q], F32, tag="lines")

        pbt = prep.tile([128, QP * 4], F32, tag="pbt")
        nc.sync.dma_start(
            pbt[:],
            pb.rearrange("b q c -> (b q c)").rearrange("(p x) -> p x", p=128),
        )
        # free layout (r,c) -> (c,r) so coord rows are contiguous per partition
        pbt2 = prep.tile([128, QP * 4], F32, tag="pbt2")
        nc.vector.tensor_scalar(
            pbt2[:].rearrange("p (c r) -> p c r", c=4),
            pbt[:].rearrange("p (r c) -> p r c", c=4).transpose([0, 2, 1]),
            0.0, None, Op.add
        )
        dx = prep.tile([128, QP], F32, tag="dx")
        dy = prep.tile([128, QP], F32, tag="dy")
        pa0 = prep.tile([128, QP], F32, tag="pa0")
        paE = prep.tile([128, QP], F32, tag="paE")
        nc.vector.tensor_tensor(dx[:], pbt2[:, 2 * QP:3 * QP], pbt2[:, 0:QP],
                                Op.subtract)
        nc.vector.tensor_tensor(dy[:], pbt2[:, 3 * QP:4 * QP], pbt2[:, QP:2 * QP],
                                Op.subtract)
        nc.vector.tensor_tensor(pa0[:], dx[:], dy[:], Op.mult)
        # fold the union's +1e-7 into the query area (union = pa+eps+ta-inter)
        nc.vector.tensor_scalar(paE[:], pa0[:], 1e-7, None, Op.add)

        # sigmoid(x) = 1 / (1 + exp(-x)); exp on ScalarE, accurate recip on DVE
        lg = prep.tile([128, QP], F32, tag="lg")
        nc.sync.dma_start(
            lg[:], pl.rearrange("b q c -> (b q c)").rearrange("(p x) -> p x", p=128)
        )
        ex = prep.tile([128, QP], F32, tag="ex")
        nc.scalar.activation(ex[:], lg[:], mybir.ActivationFunctionType.Exp,
                             scale=-1.0)
        w1 = prep.tile([128, QP], F32, tag="w1")
        nc.vector.tensor_scalar(w1[:], ex[:], 1.0, None, Op.add)
        sc = prep.tile([128, QP], F32, tag="sc")
        scr = prep.tile([128, QP], F32, tag="scr")
        nc.vector.reciprocal_approx_accurate(sc[:], w1[:], scr[:])

        # one reorg DMA each: [128, x] query-packed -> [16, 8x] image-packed
        nc.sync.dma_start(lines_all[:, 0:4 * Q], pbt2[:])
        nc.sync.dma_start(lines_all[:, 4 * Q:5 * Q], paE[:])
        nc.sync.dma_start(lines_all[:, 5 * Q:6 * Q], sc[:])

        # ---- prep: all target boxes in one DMA; areas computed on-chip
        # tsc_all[p, (b,c,k)] = tgt box k-coord of gt (c*100+p) of image b
        tsc_all = persist.tile([PCH, BPC * NCH * 4], F32, tag="tsc")
        nc.sync.dma_start(
            tsc_all[:], tb.rearrange("b (c p) k -> p b c k", c=NCH, p=PCH)
        )
        ta_all = persist.tile([PCH, BPC * NCH], F32, tag="ta")
        tdx = prep.tile([PCH, BPC * NCH], F32, tag="tdx")
        tdy = prep.tile([PCH, BPC * NCH], F32, tag="tdy")
        tv = tsc_all[:].rearrange("p (s k) -> p s k", k=4)
        nc.vector.tensor_tensor(tdx[:], tv[:, :, 2], tv[:, :, 0], Op.subtract)
        nc.vector.tensor_tensor(tdy[:], tv[:, :, 3], tv[:, :, 1], Op.subtract)
        nc.vector.tensor_tensor(ta_all[:], tdx[:], tdy[:], Op.mult)

        # ---- collectors for the whole core (written per chunk, drained once)
        v8all = persist.tile([PCH, BPC * NCH * 8], F32, tag="v8all")
        i8all = persist.tile([PCH, BPC * NCH * 8], U32, tag="i8all")
        vals4 = persist.tile([PCH, BPC * NCH * K], F32, tag="vals4")
        mask4 = persist.tile([PCH, BPC * NCH * K], U8, tag="mask4")

        HB = 500  # psum bank-sized matmul piece (N<=512)

        for _ in range(reps):
            for b in range(BPC):
                # stage this image's 6 rows on partition 0 (single-descriptor)
                stage = stagep.tile([1, 6 * Q], F32, tag="stage")
                nc.sync.dma_start(stage[:], lines_all[b:b + 1, :])
                boxv = stage[:, 0:4 * Q].rearrange(
                    "o (ph c r) -> o ph c r", ph=PH, c=4)
                pav = stage[:, 4 * Q:5 * Q].rearrange("o (ph r) -> o ph r", ph=PH)
                scv = stage[:, 5 * Q:6 * Q].rearrange("o (ph r) -> o ph r", ph=PH)

                # PE ones-matmul broadcast (bit-exact 1.0*x) + ScalarE copies
                r_px1 = rows.tile([128, Q], F32, tag="px1")
                r_py1 = rows.tile([128, Q], F32, tag="py1")
                r_px2 = rows.tile([128, Q], F32, tag="px2")
                r_py2 = rows.tile([128, Q], F32, tag="py2")
                r_pa = rows.tile([128, Q], F32, tag="pa")
                r_sc = rows.tile([128, Q], F32, tag="sc")
                views = [boxv[:, :, 0, :], boxv[:, :, 1, :], boxv[:, :, 2, :],
                         boxv[:, :, 3, :], pav, scv]
                for rt, view in zip((r_px1, r_py1, r_px2, r_py2, r_pa, r_sc),
                                    views):
                    pt = psum.tile([128, 1024], F32, tag="pt")
                    for h in range(2):
                        nc.tensor.matmul(
                            pt[:, h * 512:h * 512 + HB], ones[:],
                            view[:, 4 * h:4 * h + 4, :],
                            start=True, stop=True)
                    nc.scalar.activation(
                        rt[:].rearrange("p (h x) -> p h x", h=2),
                        pt[:].rearrange("p (h x) -> p h x", h=2)[:, :, 0:HB],
                        mybir.ActivationFunctionType.Copy)

                for c in range(NCH):
                    sb = b * NCH + c
                    ts4 = tsc_all[0:PCH, 4 * sb:4 * sb + 4]
                    tx1, ty1 = ts4[:, 0:1], ts4[:, 1:2]
                    tx2, ty2 = ts4[:, 2:3], ts4[:, 3:4]
                    ta = ta_all[0:PCH, sb:sb + 1]

                    # overlap widths; relu folded into the x op so inter
                    # is a plain Pool multiply (Pool only runs TensorTensor)
                    wxr = work.tile([PCH, Q], F32, tag="A")
                    nc.vector._custom_dve(wsubrelu, out=wxr[:], in0=r_px2[0:PCH],
                                          in1=r_px1[0:PCH], s0=tx2, s1=tx1)
                    wyr = work.tile([PCH, Q], F32, tag="B")
                    nc.vector._custom_dve(wsub, out=wyr[:], in0=r_py2[0:PCH],
                                          in1=r_py1[0:PCH], s0=ty2, s1=ty1)
                    # inter = relu(wxr) * wyr (sign-exact where it matters)
                    inter = work.tile([PCH, Q], F32, tag="C")
                    nc.gpsimd.tensor_tensor(inter[:], wxr[:], wyr[:], Op.mult)
                    # Up = (pa+eps + ta) - inter
                    Up = work.tile([PCH, Q], F32, tag="D")
                    if kb_up == "pool":
                        # ScalarE broadcasts the per-gt area into a row, the
                        # subtract stays on Pool
                        srow = work.tile([PCH, Q], F32, tag="G")
                        nc.scalar.activation(srow[:], r_pa[0:PCH],
                                             mybir.ActivationFunctionType.Identity,
                                             bias=ta)
                        nc.gpsimd.tensor_tensor(Up[:], srow[:], inter[:],
                                                Op.subtract)
                    else:
                        nc.vector.scalar_tensor_tensor(Up[:], r_pa[0:PCH], ta,
                                                       inter[:], Op.add,
                                                       Op.subtract)
                    # R ~= 1/Up to ~2 ULP (fast seed + one Newton step)
                    R0 = work.tile([PCH, Q], F32, tag="E")
                    nc.vector.reciprocal_approx_fast(out=R0[:], in_=Up[:])
                    R = work.tile([PCH, Q], F32, tag="F")
                    if kb_nr == "dve":
                        nc.vector._custom_dve(RECIPROCAL_APPROX_NR, out=R[:],
                                              in0=Up[:], in1=R0[:], s0=2.0)
                    else:
                        z1 = work.tile([PCH, Q], F32, tag="G")
                        nc.gpsimd.tensor_tensor(z1[:], Up[:], R0[:], Op.mult)
                        z2 = work.tile([PCH, Q], F32, tag="G")
                        nc.gpsimd.tensor_tensor(z2[:], c2r[0:PCH], z1[:],
                                                Op.subtract)
                        nc.gpsimd.tensor_tensor(R[:], R0[:], z2[:], Op.mult)
                    # nm = inter if inter > 0.4*Up else 0   (valid-masked)
                    nm = work.tile([PCH, Q], F32, tag="G")
                    if kb_nm == "dve":
                        nc.vector._custom_dve(masknum, out=nm[:], in0=inter[:],
                                              in1=Up[:], s0=0.4)
                    else:
                        n1 = work.tile([PCH, Q], F32, tag="H")
                        nc.gpsimd.tensor_tensor(n1[:], Up[:], c04r[0:PCH],
                                                Op.mult)
                        v01 = work.tile([PCH, Q], F32, tag="H")
                        nc.gpsimd.tensor_tensor(v01[:], inter[:], n1[:], Op.is_gt)
                        nc.gpsimd.tensor_tensor(nm[:], inter[:], v01[:], Op.mult)
                    # m3 = nm*R*score + bias
                    m1 = work.tile([PCH, Q], F32, tag="A")
                    eng = nc.gpsimd if kb_m1 == "pool" else nc.vector
                    eng.tensor_tensor(m1[:], nm[:], R[:], Op.mult)
                    t1 = work.tile([PCH, Q], F32, tag="E")
                    eng = nc.gpsimd if kb_t1 == "pool" else nc.vector
                    eng.tensor_tensor(t1[:], m1[:], r_sc[0:PCH], Op.mult)
                    m3 = work.tile([PCH, Q], F32, tag="B")
                    eng = nc.gpsimd if kb_m3 == "pool" else nc.vector
                    eng.tensor_tensor(m3[:], t1[:], bias_f[0:PCH], Op.add)

                    v8 = v8all[0:PCH, 8 * sb:8 * sb + 8]
                    nc.vector.max(v8, m3[:])
                    nc.vector.max_index(i8all[0:PCH, 8 * sb:8 * sb + 8], v8, m3[:])

            # ---- epilogue: exact zeros for padding slots + bool mask
            v8v = v8all[0:PCH, :].rearrange("p (s e) -> p s e", e=8)[:, :, 0:K]
            nc.vector.scalar_tensor_tensor(
                vals4[0:PCH, :].rearrange("p (s e) -> p s e", e=K),
                v8v, POS_THRESH, v8v, Op.is_gt, Op.mult)
            nc.vector.tensor_scalar(
                mask4[0:PCH, :].rearrange("p (s e) -> p s e", e=K),
                v8v, POS_THRESH, None, Op.is_gt)

            if kb_out == "batch":
                nc.sync.dma_start(
                    vals_o.rearrange("b (c p) k -> p b c k", c=NCH, p=PCH),
                    vals4[0:PCH, :])
                nc.sync.dma_start(
                    idxs_o.rearrange("b (c p) k -> p b c k", c=NCH, p=PCH),
                    i8all[0:PCH, :].rearrange("p (s e) -> p s e", e=8)[:, :, 0:K]
                    .bitcast(I32))
                nc.sync.dma_start(
                    mask_o.rearrange("b (c p) k -> p b c k", c=NCH, p=PCH),
                    mask4[0:PCH, :])
            else:
                for b in range(BPC):
                    for c in range(NCH):
                        sb = b * NCH + c
                        g0 = c * PCH
                        nc.sync.dma_start(
                            vals_o[b, g0:g0 + PCH, :],
                            vals4[0:PCH, K * sb:K * sb + K])
                        nc.sync.dma_start(
                            idxs_o[b, g0:g0 + PCH, :],
                            i8all[0:PCH, 8 * sb:8 * sb + K].bitcast(I32))
                        nc.sync.dma_start(
                            mask_o[b, g0:g0 + PCH, :],
                            mask4[0:PCH, K * sb:K * sb + K])

    nc.compile()
    return nc


_NC = None


def _get_nc():
    global _NC
    if _NC is None:
        _NC = _build_kernel()
    return _NC


def run(pred_logits, pred_boxes_xyxy, tgt_boxes_xyxy, **spmd_kwargs):
    nc = _get_nc()
    pred_logits = np.ascontiguousarray(np.asarray(pred_logits, dtype=np.float32))
    pred_boxes = np.ascontiguousarray(np.asarray(pred_boxes_xyxy, dtype=np.float32))
    tgt_boxes = np.ascontiguousarray(np.asarray(tgt_boxes_xyxy, dtype=np.float32))
    in_maps = [
        {
            "pred_logits": pred_logits[c * BPC:(c + 1) * BPC],
            "pred_boxes": pred_boxes[c * BPC:(c + 1) * BPC],
            "tgt_boxes": tgt_boxes[c * BPC:(c + 1) * BPC],
        }
        for c in range(NCORES)
    ]
    res = run_bass_kernel_spmd(nc, in_maps, list(range(NCORES)), **spmd_kwargs)
    vals = np.concatenate([res.results[c]["vals"] for c in range(NCORES)], axis=0)
    idxs = np.concatenate([res.results[c]["idxs"] for c in range(NCORES)], axis=0)
    mask = np.concatenate([res.results[c]["mask"] for c in range(NCORES)], axis=0)
    return (vals, idxs.astype(np.int32), mask.astype(bool)), res


def kernel(pred_logits, pred_boxes_xyxy, tgt_boxes_xyxy):
    (vals, idxs, mask), _ = run(pred_logits, pred_boxes_xyxy, tgt_boxes_xyxy)
    return vals, idxs, mask
